# revision 1
# baseline (speedup 1.0000x reference)
"""MoE minGRU layer for Trainium2, 8 NeuronCores.

Problem: nn_MoEMinGRULayer (B=4, S=2048, D=1024, M=4 experts, top-2 router).

Sharding: expert-parallel (4) x batch-parallel (2). Core c handles expert
m = c//2 and batches [2*(c%2), 2*(c%2)+1]. Each core computes its expert's
dense g/v/d projections for its 4096 tokens, the minGRU recurrence (a native
DVE tensor_tensor_scan along the sequence), and the router weight for its
own expert (gate_W columns are permuted host-side so column 0 is always
"this core's expert"). The core returns w_m(t) * h_m(t, :); the host sums
the 4 expert partials per batch group.

Layout on chip: activations are kept as [d_model on partitions, tokens on
free] so the scan can run along the free dim; x is transposed once via the
PE, and h is transposed back before the weighted store. Matmuls run in
float32r (full bf16-rate, ~1e-4 relative error).
"""

import os
import numpy as np

MM_DT = os.environ.get("KERNEL_MM_DT", "f16h")  # f16h|f32rh|f16x|f32r|f16

B, S, D, M = 4, 2048, 1024, 4
T = 2 * S            # tokens per core (2 batches)
KC = D // 128        # contraction chunks
ET = D // 128        # expert-dim tiles
TCH = 512            # tokens per chunk
NCH = T // TCH       # chunks per core
JT = TCH // 128      # 128-token subtiles per chunk
CH_PER_SEQ = S // TCH  # chunks per sequence (scan restarts here)

LAST_RESULT = None   # BassKernelResults of the most recent run (for test.py)
_PROG_CACHE = {}


def _build_program(reps=0, mm_dt=None):
    mm_dt = mm_dt or MM_DT
    from contextlib import ExitStack

    import concourse.bacc as bacc
    import concourse.mybir as mybir
    import concourse.tile as tile
    from concourse.masks import make_identity

    F32 = mybir.dt.float32
    F16 = mybir.dt.float16
    F32R = mybir.dt.float32r
    # WDT: expert-weight / big-matmul dtype. XTDT: transposed-x dtype used by
    # the router (and by the big matmuls unless f16x casts a second copy).
    # GWDT: gate-weight dtype (must match XTDT family).
    host_t = mm_dt in ("f16h", "f32rh")
    WDT = F16 if mm_dt in ("f16", "f16x", "f16h") else F32R
    XDT = F16 if mm_dt == "f16" else F32
    XTDT = F16 if mm_dt == "f16" else F32R
    GWDT = F16 if mm_dt == "f16" else F32R
    AF = mybir.ActivationFunctionType
    OP = mybir.AluOpType

    nc = bacc.Bacc("TRN2", target_bir_lowering=False)

    if host_t:
        x_d = nc.declare_dram_parameter("x", [D, T], F32R, isOutput=False)
        if mm_dt == "f16h":
            x16_d = nc.declare_dram_parameter("x16", [D, T], F16, isOutput=False)
    else:
        x_d = nc.declare_dram_parameter("x", [T, D], XDT, isOutput=False)
    wg_d = nc.declare_dram_parameter("wg", [D, D], WDT, isOutput=False)
    wv_d = nc.declare_dram_parameter("wv", [D, D], WDT, isOutput=False)
    wd_d = nc.declare_dram_parameter("wd", [D, D], WDT, isOutput=False)
    bg_d = nc.declare_dram_parameter("bg", [D], F32, isOutput=False)
    bv_d = nc.declare_dram_parameter("bv", [D], F32, isOutput=False)
    bd_d = nc.declare_dram_parameter("bd", [D], F32, isOutput=False)
    gw_d = nc.declare_dram_parameter("gw", [D, M], GWDT, isOutput=False)
    out_d = nc.declare_dram_parameter("out", [T, D], F32, isOutput=True)

    with ExitStack() as ctx:
        tc = ctx.enter_context(tile.TileContext(nc))
        consts = ctx.enter_context(tc.tile_pool(name="consts", bufs=1))
        wpool = ctx.enter_context(tc.tile_pool(name="w", bufs=1))
        xload = ctx.enter_context(tc.tile_pool(name="xload", bufs=2))
        xtp = ctx.enter_context(tc.tile_pool(name="xt", bufs=1))
        inter = ctx.enter_context(tc.tile_pool(name="inter", bufs=2))
        hpool = ctx.enter_context(tc.tile_pool(name="h", bufs=12))
        carryp = ctx.enter_context(tc.tile_pool(name="carry", bufs=2))
        outst = ctx.enter_context(tc.tile_pool(name="outst", bufs=2))
        routp = ctx.enter_context(tc.tile_pool(name="rout", bufs=2))
        psmm = ctx.enter_context(tc.tile_pool(name="psmm", bufs=2, space="PSUM"))
        pstr = ctx.enter_context(tc.tile_pool(name="pstr", bufs=2, space="PSUM"))

        ident = consts.tile([128, 128], F32, tag="ident", name="ident")
        make_identity(nc, ident)
        if XDT is F32:
            ident_x = ident
        else:
            ident_x = consts.tile([128, 128], XDT, tag="identx", name="identx")
            make_identity(nc, ident_x)

        # Small tensors first so chunk-0 transposes/router start immediately
        # while the big weight DMAs stream in behind them.
        gw_sb = consts.tile([128, KC, M], GWDT, tag="gw", name="gw")
        for kc in range(KC):
            nc.sync.dma_start(out=gw_sb[:, kc, :], in_=gw_d[kc * 128:(kc + 1) * 128, :])

        # Biases: [e on partitions within tile, et tile index on free]
        b_sb = {}
        for nm, dram in (("bg", bg_d), ("bv", bv_d), ("bd", bd_d)):
            t = consts.tile([128, ET], F32, tag=nm + "s", name=nm + "s")
            nc.sync.dma_start(out=t, in_=dram[:].rearrange("(et p) -> p et", p=128))
            b_sb[nm] = t

        def load_x(ch):
            t0 = ch * TCH
            row = []
            for j in range(JT):
                xt = xload.tile([128, D], XDT, tag=f"xl{j}", name=f"xl{j}")
                nc.sync.dma_start(out=xt, in_=x_d[t0 + j * 128: t0 + (j + 1) * 128, :])
                row.append(xt)
            return row

        def load_xt(ch):
            """Host-transposed path: one 3D-AP DMA per dtype per chunk."""
            t0 = ch * TCH
            xT = xtp.tile([128, KC, TCH], F32R, tag="xT", name="xT", bufs=2)
            nc.sync.dma_start(
                out=xT,
                in_=x_d[:, t0:t0 + TCH].rearrange("(kc p) t -> p kc t", p=128))
            if mm_dt == "f16h":
                xT16 = xtp.tile([128, KC, TCH], F16, tag="xT16", name="xT16", bufs=2)
                nc.sync.dma_start(
                    out=xT16,
                    in_=x16_d[:, t0:t0 + TCH].rearrange("(kc p) t -> p kc t", p=128))
            else:
                xT16 = xT
            return xT, xT16

        # x chunk 0 before the big weight DMAs so chunk-0 compute starts
        # immediately; weights stream in behind.
        if host_t:
            xt_next = load_xt(0)
        else:
            xrow_next = load_x(0)

        # Expert weights: [d_in on partitions (kc chunks), d_out on free].
        # wg first: the et-loop issues all g-matmuls before v/d, so compute
        # can start as soon as wg lands.
        w_sb = {}
        for nm, dram in (("wg", wg_d), ("wv", wv_d), ("wd", wd_d)):
            t = wpool.tile([128, KC, D], WDT, tag=nm, name=nm)
            for kc in range(KC):
                nc.sync.dma_start(out=t[:, kc, :], in_=dram[kc * 128:(kc + 1) * 128, :])
            w_sb[nm] = t

        rep_ctx = ctx.enter_context(tc.For_i(0, reps, 1)) if reps else None  # noqa: F841

        osb_cur = []

        def out_stage(ch, et, h, w_t):
            """Transpose h back to [token, e], scale by the router weight into
            the per-chunk assembly tiles; store contiguously after et=7 (4KB
            rows -- the 512B-run per-et store pattern measured +160us/rep)."""
            t0 = ch * TCH
            es = slice(et * 128, (et + 1) * 128)
            if et == 0:
                osb_cur.clear()
                for j in range(JT):
                    osb_cur.append(outst.tile([128, D], F32, tag=f"ob{j}", name=f"ob{j}"))
            pto = pstr.tile([128, TCH], F32, tag="tr", name="tr")
            for j in range(JT):
                nc.tensor.transpose(pto[:, j * 128:(j + 1) * 128],
                                    h[:, j * 128:(j + 1) * 128], ident)
            for j in range(JT):
                if et % 2 == 0:
                    nc.vector.tensor_scalar_mul(osb_cur[j][:, es],
                                                pto[:, j * 128:(j + 1) * 128],
                                                w_t[:, j:j + 1])
                else:
                    nc.scalar.activation(osb_cur[j][:, es], pto[:, j * 128:(j + 1) * 128],
                                         AF.Copy, bias=0.0, scale=w_t[:, j:j + 1])
            if et == ET - 1:
                for j in range(JT):
                    nc.sync.dma_start(
                        out=out_d[t0 + j * 128:t0 + (j + 1) * 128, :],
                        in_=osb_cur[j])

        hcarry = [None] * ET
        h_prev = None
        w_prev = None
        for ch in range(NCH):
            seq_start = (ch % CH_PER_SEQ == 0)

            if host_t:
                xT, xT16 = xt_next
                if ch + 1 < NCH:
                    xt_next = load_xt(ch + 1)
            else:
                xrow = xrow_next
                if ch + 1 < NCH:
                    xrow_next = load_x(ch + 1)

                # Transpose to xT: [d on partitions (kc chunks), tokens on free]
                xT = xtp.tile([128, KC, TCH], XTDT, tag="xT", name="xT")
                xT16 = (xtp.tile([128, KC, TCH], F16, tag="xT16", name="xT16", bufs=2)
                        if mm_dt == "f16x" else xT)
                for kc in range(KC):
                    pst = pstr.tile([128, TCH], XDT, tag="tr", name="tr")
                    for j in range(JT):
                        nc.tensor.transpose(
                            pst[:, j * 128:(j + 1) * 128],
                            xrow[j][:, kc * 128:(kc + 1) * 128], ident_x)
                    for j in range(JT):
                        dst = xT[:, kc, j * 128:(j + 1) * 128]
                        src = pst[:, j * 128:(j + 1) * 128]
                        if kc % 2 == 0:
                            nc.scalar.copy(dst, src)
                        else:
                            nc.vector.tensor_copy(dst, src)
                    if mm_dt == "f16x":
                        nc.vector.tensor_copy(xT16[:, kc, :], xT[:, kc, :].bitcast(F32))

            # Expert projections + minGRU scan; the PREVIOUS chunk's output
            # stage is interleaved here so its h-transposes hide inside the
            # matmul spans (its router weights are long since ready).
            h_tiles = []
            for et in range(ET):
                pg = psmm.tile([128, TCH], F32, tag="pg", name="pg")
                pv = psmm.tile([128, TCH], F32, tag="pv", name="pv")
                pd = psmm.tile([128, TCH], F32, tag="pd", name="pd")
                es = slice(et * 128, (et + 1) * 128)
                for ps, wn in ((pg, "wg"), (pv, "wv"), (pd, "wd")):
                    for kc in range(KC):
                        nc.tensor.matmul(ps, w_sb[wn][:, kc, es], xT16[:, kc, :],
                                         start=(kc == 0), stop=(kc == KC - 1))
                gs = inter.tile([128, TCH], F32, tag="gs", name="gs")
                vt = inter.tile([128, TCH], F32, tag="vt", name="vt")
                aa = inter.tile([128, TCH], F32, tag="aa", name="aa")
                nc.scalar.activation(gs, pg, AF.Sigmoid, bias=b_sb["bg"][:, et:et + 1])
                nc.scalar.activation(vt, pv, AF.Tanh, bias=b_sb["bv"][:, et:et + 1])
                nc.scalar.activation(aa, pd, AF.Sigmoid, bias=b_sb["bd"][:, et:et + 1])
                nc.vector.tensor_scalar(aa, aa, 0.998, 0.001, OP.mult, OP.add)
                nc.vector.tensor_tensor(gs, gs, vt, OP.mult)   # x_scan, in place
                h = hpool.tile([128, TCH], F32, tag="h", name="h")
                init = 0.0 if seq_start else hcarry[et][:, 0:1]
                nc.vector.tensor_tensor_scan(h, aa, gs, init, OP.mult, OP.add)
                nhc = carryp.tile([128, 1], F32, tag=f"c{et}", name=f"c{et}")
                nc.vector.tensor_copy(nhc, h[:, TCH - 1:TCH])
                hcarry[et] = nhc
                h_tiles.append(h)
                if h_prev is not None:
                    out_stage(ch - 1, et, h_prev[et], w_prev)

            # Router: gate logits [tokens on partitions, expert on free].
            # Emitted after the et-loop: these matmuls re-read the already
            # resident xT, so the chunk boundary never stalls on a full xT.
            gate_ps = pstr.tile([128, JT * M], F32, tag="tr", name="tr")
            for j in range(JT):
                for kc in range(KC):
                    nc.tensor.matmul(
                        gate_ps[:, j * M:(j + 1) * M],
                        xT[:, kc, j * 128:(j + 1) * 128],
                        gw_sb[:, kc, :],
                        start=(kc == 0), stop=(kc == KC - 1))
            l_sb = routp.tile([128, JT * M], F32, tag="l", name="l")
            nc.scalar.copy(l_sb, gate_ps)
            # top-2-of-4 softmax weight of expert column 0, batched over j:
            #   w0 = 1{l0 >= mx2} * sigmoid(2*l0 - mx1 - mx2)
            #   mx1 = max of 4; mx2 = max(min(m1,m2), max(n1,n2)) with
            #   m/n = pairwise max/min.
            l3 = l_sb.rearrange("p (j m) -> p j m", m=M)
            pa, pb = l3[:, :, 0:2], l3[:, :, 2:4]

            def rt(tag, free=JT):
                return routp.tile([128, free], F32, tag=tag, name=tag)

            def v3(t):  # [128, JT] tile -> [128, JT, 1] view
                return t.rearrange("p (j o) -> p j o", o=1)

            mp, mn = rt("mp", JT * 2), rt("mn", JT * 2)
            mp3 = mp.rearrange("p (j m) -> p j m", m=2)
            mn3 = mn.rearrange("p (j m) -> p j m", m=2)
            nc.vector.tensor_tensor(mp3, pa, pb, OP.max)
            nc.vector.tensor_tensor(mn3, pa, pb, OP.min)
            mx1, mmn, nmx, mx2 = rt("mx1"), rt("mmn"), rt("nmx"), rt("mx2")
            nc.vector.tensor_tensor(v3(mx1), mp3[:, :, 0:1], mp3[:, :, 1:2], OP.max)
            nc.vector.tensor_tensor(v3(mmn), mp3[:, :, 0:1], mp3[:, :, 1:2], OP.min)
            nc.vector.tensor_tensor(v3(nmx), mn3[:, :, 0:1], mn3[:, :, 1:2], OP.max)
            nc.vector.tensor_tensor(mx2, mmn, nmx, OP.max)
            ssum, tt = rt("ssum"), rt("tt")
            nc.vector.tensor_tensor(ssum, mx1, mx2, OP.add)
            nc.vector.scalar_tensor_tensor(v3(tt), l3[:, :, 0:1], 2.0, v3(ssum),
                                           OP.mult, OP.subtract)
            sg, ind, w_t = rt("sg"), rt("ind"), rt("wt")
            nc.scalar.activation(sg, tt, AF.Sigmoid)
            nc.vector.tensor_tensor(v3(ind), l3[:, :, 0:1], v3(mx2), OP.is_ge)
            nc.vector.tensor_tensor(w_t, ind, sg, OP.mult)

            h_prev, w_prev = h_tiles, w_t

        # Flush the last chunk's output stage.
        for et in range(ET):
            out_stage(NCH - 1, et, h_prev[et], w_prev)

    nc.compile()
    return nc


def _get_program():
    if "nc" not in _PROG_CACHE:
        _PROG_CACHE["nc"] = _build_program()
    return _PROG_CACHE["nc"]


def kernel(x, Wg, bg, Wv, bv, Wd, bd, gate_W):
    global LAST_RESULT
    from concourse.bass_utils import run_bass_kernel_spmd

    f = np.float32
    host_t = MM_DT in ("f16h", "f32rh")
    wdt = np.float16 if MM_DT in ("f16", "f16x", "f16h") else f
    xdt = np.float16 if MM_DT == "f16" else f
    gwdt = np.float16 if MM_DT == "f16" else f
    x = np.ascontiguousarray(np.asarray(x, f).astype(xdt))
    Wg, Wv, Wd = (np.asarray(a, f).astype(wdt) for a in (Wg, Wv, Wd))
    bg, bv, bd = (np.asarray(a, f) for a in (bg, bv, bd))
    gate_W = np.asarray(gate_W, f).astype(gwdt)

    nc = _get_program()
    in_maps = []
    for c in range(8):
        m, grp = divmod(c, 2)
        perm = [m] + [e for e in range(M) if e != m]
        im = {}
        if host_t:
            xt = np.ascontiguousarray(x[2 * grp:2 * grp + 2].reshape(T, D).T)
            im["x"] = xt
            if MM_DT == "f16h":
                im["x16"] = np.ascontiguousarray(xt.astype(np.float16))
        else:
            im["x"] = np.ascontiguousarray(x[2 * grp:2 * grp + 2].reshape(T, D))
        in_maps.append({
            **im,
            "wg": np.ascontiguousarray(Wg[m]),
            "wv": np.ascontiguousarray(Wv[m]),
            "wd": np.ascontiguousarray(Wd[m]),
            "bg": np.ascontiguousarray(bg[m]),
            "bv": np.ascontiguousarray(bv[m]),
            "bd": np.ascontiguousarray(bd[m]),
            "gw": np.ascontiguousarray(gate_W[:, perm]),
        })

    trace = bool(int(os.environ.get("KERNEL_TRACE", "0")))
    res = _run(nc, in_maps, trace)

    out = np.zeros((B, S, D), f)
    for c in range(8):
        m, grp = divmod(c, 2)
        out[2 * grp:2 * grp + 2] += res[c]["out"].reshape(2, S, D)
    return out


def _make_runner(nc, n_cores=8):
    """Cached jitted shard_map executor (mirrors run_bass_kernel_spmd's axon
    path, but reusable across calls: no re-trace / re-jit / re-compile)."""
    import jax
    from jax.sharding import Mesh, PartitionSpec
    from jax.experimental.shard_map import shard_map
    import concourse.mybir as mybir
    from concourse import bass2jax

    bass2jax.install_neuronx_cc_hook()
    pname = nc.partition_id_tensor.name if nc.partition_id_tensor else None
    in_names, out_names, out_avals = [], [], []
    for alloc in nc.m.functions[0].allocations:
        if not isinstance(alloc, mybir.MemoryLocationSet):
            continue
        name = alloc.memorylocations[0].name
        if alloc.kind == "ExternalInput":
            if name != pname:
                in_names.append(name)
        elif alloc.kind == "ExternalOutput":
            out_names.append(name)
            out_avals.append(jax.core.ShapedArray(
                tuple(alloc.tensor_shape), mybir.dt.np(alloc.dtype)))
    n_params = len(in_names)
    all_in_names = in_names + out_names + ([pname] if pname else [])

    def _body(*args):
        operands = list(args)
        if pname is not None:
            operands.append(bass2jax.partition_id_tensor())
        return tuple(bass2jax._bass_exec_p.bind(
            *operands,
            out_avals=tuple(out_avals),
            in_names=tuple(all_in_names),
            out_names=tuple(out_names),
            lowering_input_output_aliases=(),
            sim_require_finite=True,
            sim_require_nnan=True,
            nc=nc,
        ))

    devices = jax.devices()[:n_cores]
    mesh = Mesh(np.asarray(devices), ("core",))
    nspecs = n_params + len(out_names)
    fn = jax.jit(shard_map(_body,
                           mesh=mesh,
                           in_specs=(PartitionSpec("core"),) * nspecs,
                           out_specs=(PartitionSpec("core"),) * len(out_names),
                           check_rep=False))
    return fn, in_names, out_names, out_avals, mesh


def _run(nc, in_maps, trace=False):
    try:
        import jax
        from jax.sharding import PartitionSpec, NamedSharding
        if "runner" not in _PROG_CACHE:
            _PROG_CACHE["runner"] = _make_runner(nc)
        fn, in_names, out_names, out_avals, mesh = _PROG_CACHE["runner"]
        n = len(in_maps)
        sh = NamedSharding(mesh, PartitionSpec("core"))
        args = [jax.device_put(
            np.concatenate([in_maps[c][nm] for c in range(n)], axis=0), sh)
            for nm in in_names]
        args += [jax.device_put(
            np.zeros((n * a.shape[0], *a.shape[1:]), a.dtype), sh)
            for a in out_avals]
        outs = jax.block_until_ready(fn(*args))
        return [{nm: np.asarray(outs[i]).reshape(n, *out_avals[i].shape)[c]
                 for i, nm in enumerate(out_names)}
                for c in range(n)]
    except Exception:
        from concourse.bass_utils import run_bass_kernel_spmd
        global LAST_RESULT
        res = run_bass_kernel_spmd(nc, in_maps, core_ids=list(range(len(in_maps))),
                                   trace=trace)
        LAST_RESULT = res
        return res.results



# revision 2
# speedup vs baseline: 6.9615x; 6.9615x over previous
"""MoE minGRU layer for Trainium2, 8 NeuronCores.

Problem: nn_MoEMinGRULayer (B=4, S=2048, D=1024, M=4 experts, top-2 router).

The end-to-end wall clock of kernel() is dominated by the host<->device
tunnel (~50-100 MB/s), so the design minimizes transferred bytes:

- Router (top-2 softmax combine weights) computed on host in f32 (matches
  the reference's selection exactly); only the per-expert combine weight
  w_m(t) [T] is shipped per core (16 KB) instead of an x copy for on-device
  gate logits.
- x is cast to f16 and shipped exactly ONCE (16.8 MB total): each core gets
  a 1/8 shard (a transposed [D, 1024-token] slab) and an on-device
  AllGather over the 4 cores of its batch group reconstitutes the full
  [D, 4096-token] activation (on-chip links are ~1000x faster than the
  tunnel).
- Expert weights f16 are shipped exactly once (25.2 MB total): each core
  gets half of its expert's stacked [3D, D] weights; a pairwise AllGather
  reconstitutes them.
- The masked combine sum_m w_m(t) h_m(t) is reduced ON DEVICE with an f16
  ReduceScatter over each batch group, so D2H is the final output, f16,
  shipped exactly once (16.8 MB total).
- The output zero-init operand is input-independent and cached on device
  across calls.

Core c handles expert m = c//2 and batch group grp = c%2 (batches
[2*grp, 2*grp+1], T = 4096 tokens). On chip, activations are kept as
[d_model on partitions, tokens on free] so the minGRU recurrence runs as a
native DVE tensor_tensor_scan along the free dim; h is transposed back via
the PE before the weighted store. Matmuls run in f16 (~3e-4 rel err).
"""

import os
import numpy as np

B, S, D, M = 4, 2048, 1024, 4
T = 2 * S            # tokens per batch group (2 batches)
TQ = T // 4          # tokens per core shard (AG/RS quarter)
KC = D // 128        # contraction chunks
ET = D // 128        # expert-dim tiles
TCH = 512            # tokens per chunk
NCH = T // TCH       # chunks per core
JT = TCH // 128      # 128-token subtiles per chunk
CH_PER_SEQ = S // TCH  # chunks per sequence (scan restarts here)

# core c = 2*m + grp; slab s = 4*grp + m (slab = contiguous [1024,1024]
# block of x.reshape(8,1024,1024))
SLAB_OF_CORE = [4 * (c % 2) + c // 2 for c in range(8)]
CORE_OF_SLAB = [2 * (s % 4) + s // 4 for s in range(8)]

G4 = [[0, 2, 4, 6], [1, 3, 5, 7]]    # batch groups (AG x, RS out)
G2 = [[0, 1], [2, 3], [4, 5], [6, 7]]  # expert pairs (AG weights)

LAST_RESULT = None   # BassKernelResults of the most recent traced run
_PROG_CACHE = {}


def _build_program():
    from contextlib import ExitStack

    import concourse.bacc as bacc
    import concourse.mybir as mybir
    import concourse.tile as tile
    from concourse.masks import make_identity

    F32 = mybir.dt.float32
    F16 = mybir.dt.float16
    AF = mybir.ActivationFunctionType
    OP = mybir.AluOpType

    nc = bacc.Bacc("TRN2", target_bir_lowering=False)

    xs_d = nc.declare_dram_parameter("xs", [D, TQ], F16, isOutput=False)
    wh_d = nc.declare_dram_parameter("wh", [3 * D // 2, D], F16, isOutput=False)
    bias_d = nc.declare_dram_parameter("bias", [3, D], F32, isOutput=False)
    wt_d = nc.declare_dram_parameter("wt", [T], F32, isOutput=False)
    out_d = nc.declare_dram_parameter("out", [TQ, D], F16, isOutput=True)

    with ExitStack() as ctx:
        tc = ctx.enter_context(tile.TileContext(nc))
        dram = ctx.enter_context(tc.tile_pool(name="dram", bufs=1, space="DRAM"))
        consts = ctx.enter_context(tc.tile_pool(name="consts", bufs=1))
        wpool = ctx.enter_context(tc.tile_pool(name="w", bufs=1))
        xtp = ctx.enter_context(tc.tile_pool(name="xt", bufs=2))
        inter = ctx.enter_context(tc.tile_pool(name="inter", bufs=2))
        hpool = ctx.enter_context(tc.tile_pool(name="h", bufs=12))
        carryp = ctx.enter_context(tc.tile_pool(name="carry", bufs=2))
        outst = ctx.enter_context(tc.tile_pool(name="outst", bufs=2))
        psmm = ctx.enter_context(tc.tile_pool(name="psmm", bufs=2, space="PSUM"))
        pstr = ctx.enter_context(tc.tile_pool(name="pstr", bufs=2, space="PSUM"))

        # --- collectives: reconstitute x (batch group) and W (expert pair)
        xb = dram.tile([D, TQ], F16, name="xb")
        xg = dram.tile([4 * D, TQ], F16, name="xg")      # 4 slabs [D, TQ]
        wb = dram.tile([3 * D // 2, D], F16, name="wb")
        wgf = dram.tile([3 * D, D], F16, name="wgf")     # [Wg; Wv; Wd]
        rsin = dram.tile([T, D], F16, name="rsin")
        rso = dram.tile([TQ, D], F16, name="rso")

        nc.gpsimd.dma_start(out=xb[:], in_=xs_d[:])
        nc.gpsimd.collective_compute(
            "AllGather", mybir.AluOpType.bypass, replica_groups=G4,
            ins=[xb.opt()], outs=[xg.opt()])
        nc.gpsimd.dma_start(out=wb[:], in_=wh_d[:])
        nc.gpsimd.collective_compute(
            "AllGather", mybir.AluOpType.bypass, replica_groups=G2,
            ins=[wb.opt()], outs=[wgf.opt()])

        ident = consts.tile([128, 128], F32, tag="ident", name="ident")
        make_identity(nc, ident)

        # biases: [e on partitions, et tile index on free]
        b_sb = {}
        for gi, nm in enumerate(("bg", "bv", "bd")):
            t = consts.tile([128, ET], F32, tag=nm + "s", name=nm + "s")
            nc.sync.dma_start(out=t, in_=bias_d[gi].rearrange("(et p) -> p et", p=128))
            b_sb[nm] = t

        # router combine weight for this core's expert: [token%128 on
        # partitions, token//128 on free]
        wt_sb = consts.tile([128, T // 128], F32, tag="wt", name="wt")
        nc.sync.dma_start(out=wt_sb, in_=wt_d[:].rearrange("(j p) -> p j", p=128))

        # expert weights into SBUF: [d_in on partitions (kc chunks), d_out]
        w_sb = {}
        for gi, nm in enumerate(("wg", "wv", "wd")):
            t = wpool.tile([128, KC, D], F16, tag=nm, name=nm)
            nc.sync.dma_start(
                out=t,
                in_=wgf[gi * D:(gi + 1) * D, :].rearrange("(kc p) e -> p kc e", p=128))
            w_sb[nm] = t

        def load_xt(ch):
            """One 3D-AP DMA: slab q of xg, 512-token half -> [128, KC, TCH]."""
            q, half = divmod(ch, 2)
            t0c = half * TCH
            xT = xtp.tile([128, KC, TCH], F16, tag="xT", name="xT")
            nc.sync.dma_start(
                out=xT,
                in_=xg[q * D:(q + 1) * D, t0c:t0c + TCH].rearrange(
                    "(kc p) t -> p kc t", p=128))
            return xT

        xt_next = load_xt(0)

        osb_cur = []

        def out_stage(ch, et, h):
            """Transpose h back to [token, e], scale by the router weight into
            per-chunk assembly tiles; store contiguously after et=7."""
            t0 = ch * TCH
            es = slice(et * 128, (et + 1) * 128)
            if et == 0:
                osb_cur.clear()
                for j in range(JT):
                    osb_cur.append(outst.tile([128, D], F16, tag=f"ob{j}", name=f"ob{j}"))
            pto = pstr.tile([128, TCH], F32, tag="tr", name="tr")
            for j in range(JT):
                nc.tensor.transpose(pto[:, j * 128:(j + 1) * 128],
                                    h[:, j * 128:(j + 1) * 128], ident)
            for j in range(JT):
                jg = ch * JT + j
                if et % 2 == 0:
                    nc.vector.tensor_scalar_mul(osb_cur[j][:, es],
                                                pto[:, j * 128:(j + 1) * 128],
                                                wt_sb[:, jg:jg + 1])
                else:
                    nc.scalar.activation(osb_cur[j][:, es], pto[:, j * 128:(j + 1) * 128],
                                         AF.Copy, bias=0.0, scale=wt_sb[:, jg:jg + 1])
            if et == ET - 1:
                for j in range(JT):
                    nc.sync.dma_start(
                        out=rsin[t0 + j * 128:t0 + (j + 1) * 128, :],
                        in_=osb_cur[j])

        hcarry = [None] * ET
        h_prev = None
        for ch in range(NCH):
            seq_start = (ch % CH_PER_SEQ == 0)
            xT16 = xt_next
            if ch + 1 < NCH:
                xt_next = load_xt(ch + 1)

            # Expert projections + minGRU scan; the PREVIOUS chunk's output
            # stage is interleaved so its h-transposes hide in matmul spans.
            h_tiles = []
            for et in range(ET):
                pg = psmm.tile([128, TCH], F32, tag="pg", name="pg")
                pv = psmm.tile([128, TCH], F32, tag="pv", name="pv")
                pd = psmm.tile([128, TCH], F32, tag="pd", name="pd")
                es = slice(et * 128, (et + 1) * 128)
                for ps, wn in ((pg, "wg"), (pv, "wv"), (pd, "wd")):
                    for kc in range(KC):
                        nc.tensor.matmul(ps, w_sb[wn][:, kc, es], xT16[:, kc, :],
                                         start=(kc == 0), stop=(kc == KC - 1))
                gs = inter.tile([128, TCH], F32, tag="gs", name="gs")
                vt = inter.tile([128, TCH], F32, tag="vt", name="vt")
                aa = inter.tile([128, TCH], F32, tag="aa", name="aa")
                nc.scalar.activation(gs, pg, AF.Sigmoid, bias=b_sb["bg"][:, et:et + 1])
                nc.scalar.activation(vt, pv, AF.Tanh, bias=b_sb["bv"][:, et:et + 1])
                nc.scalar.activation(aa, pd, AF.Sigmoid, bias=b_sb["bd"][:, et:et + 1])
                nc.vector.tensor_scalar(aa, aa, 0.998, 0.001, OP.mult, OP.add)
                nc.vector.tensor_tensor(gs, gs, vt, OP.mult)   # x_scan, in place
                h = hpool.tile([128, TCH], F32, tag="h", name="h")
                init = 0.0 if seq_start else hcarry[et][:, 0:1]
                nc.vector.tensor_tensor_scan(h, aa, gs, init, OP.mult, OP.add)
                nhc = carryp.tile([128, 1], F32, tag=f"c{et}", name=f"c{et}")
                nc.vector.tensor_copy(nhc, h[:, TCH - 1:TCH])
                hcarry[et] = nhc
                h_tiles.append(h)
                if h_prev is not None:
                    out_stage(ch - 1, et, h_prev[et])
            h_prev = h_tiles

        # Flush the last chunk's output stage.
        for et in range(ET):
            out_stage(NCH - 1, et, h_prev[et])

        # Masked combine: sum the 4 expert partials of this batch group on
        # device; rank position m keeps token rows [TQ*m, TQ*(m+1)).
        nc.gpsimd.collective_compute(
            "ReduceScatter", mybir.AluOpType.add, replica_groups=G4,
            ins=[rsin.opt()], outs=[rso.opt()])
        nc.gpsimd.dma_start(out=out_d[:], in_=rso[:])

    nc.compile()
    return nc


def _get_program():
    if "nc" not in _PROG_CACHE:
        _PROG_CACHE["nc"] = _build_program()
    return _PROG_CACHE["nc"]


def _host_router(x2d, gate_W):
    """Top-2-of-4 softmax combine weights, f32 (matches reference top_k)."""
    f = np.float32
    logits = x2d @ np.asarray(gate_W, f)         # [N, M]
    n = logits.shape[0]
    ar = np.arange(n)
    idx1 = np.argmax(logits, axis=1)
    l1 = logits[ar, idx1]
    tmp = logits.copy()
    tmp[ar, idx1] = -np.inf
    idx2 = np.argmax(tmp, axis=1)
    l2 = tmp[ar, idx2]
    e = np.exp(l2 - l1)
    w1 = 1.0 / (1.0 + e)
    comb = np.zeros((n, M), f)
    comb[ar, idx1] = w1
    comb[ar, idx2] = w1 * e
    return comb


def kernel(x, Wg, bg, Wv, bv, Wd, bd, gate_W):
    global LAST_RESULT
    f = np.float32

    x = np.asarray(x, f)
    comb = _host_router(x.reshape(-1, D), gate_W)        # [B*S, M]

    # per-core x shards: slab s = x.reshape(8,1024,1024)[s].T, core order
    x16 = x.astype(np.float16).reshape(8, TQ, D)
    xs_cat = np.ascontiguousarray(
        x16[SLAB_OF_CORE].transpose(0, 2, 1)).reshape(8 * D, TQ)

    # per-core weight halves: concat over c equals [Wg0;Wv0;Wd0;Wg1;...]
    wh_cat = np.stack([np.asarray(Wg, f), np.asarray(Wv, f),
                       np.asarray(Wd, f)], axis=1).astype(np.float16)
    wh_cat = wh_cat.reshape(8 * (3 * D // 2), D)

    bias_cat = np.stack([np.asarray(bg, f), np.asarray(bv, f),
                         np.asarray(bd, f)], axis=1)     # [M, 3, D]
    bias_cat = bias_cat[np.repeat(np.arange(M), 2)].reshape(24, D)

    comb3 = comb.reshape(2, T, M)                        # [grp, t, m]
    wt_cat = np.ascontiguousarray(
        np.stack([comb3[c % 2, :, c // 2] for c in range(8)])).reshape(8 * T)

    nc = _get_program()
    in_cat = {"xs": xs_cat, "wh": wh_cat, "bias": bias_cat, "wt": wt_cat}

    trace = bool(int(os.environ.get("KERNEL_TRACE", "0")))
    res = _run(nc, in_cat, trace)                        # [8, TQ, D] f16

    out = res[CORE_OF_SLAB].reshape(B, S, D).astype(f)
    return out


def _make_runner(nc, n_cores=8):
    """Cached jitted shard_map executor (mirrors run_bass_kernel_spmd's axon
    path, but reusable across calls: no re-trace / re-jit / re-compile)."""
    import jax
    from jax.sharding import Mesh, PartitionSpec
    from jax.experimental.shard_map import shard_map
    import concourse.mybir as mybir
    from concourse import bass2jax

    bass2jax.install_neuronx_cc_hook()
    pname = nc.partition_id_tensor.name if nc.partition_id_tensor else None
    in_names, out_names, out_avals = [], [], []
    for alloc in nc.m.functions[0].allocations:
        if not isinstance(alloc, mybir.MemoryLocationSet):
            continue
        name = alloc.memorylocations[0].name
        if alloc.kind == "ExternalInput":
            if name != pname:
                in_names.append(name)
        elif alloc.kind == "ExternalOutput":
            out_names.append(name)
            out_avals.append(jax.core.ShapedArray(
                tuple(alloc.tensor_shape), mybir.dt.np(alloc.dtype)))
    n_params = len(in_names)
    all_in_names = in_names + out_names + ([pname] if pname else [])

    def _body(*args):
        operands = list(args)
        if pname is not None:
            operands.append(bass2jax.partition_id_tensor())
        return tuple(bass2jax._bass_exec_p.bind(
            *operands,
            out_avals=tuple(out_avals),
            in_names=tuple(all_in_names),
            out_names=tuple(out_names),
            lowering_input_output_aliases=(),
            sim_require_finite=True,
            sim_require_nnan=True,
            nc=nc,
        ))

    devices = jax.devices()[:n_cores]
    mesh = Mesh(np.asarray(devices), ("core",))
    nspecs = n_params + len(out_names)
    fn = jax.jit(shard_map(_body,
                           mesh=mesh,
                           in_specs=(PartitionSpec("core"),) * nspecs,
                           out_specs=(PartitionSpec("core"),) * len(out_names),
                           check_rep=False))
    return fn, in_names, out_names, out_avals, mesh


def _run(nc, in_cat, trace=False):
    """Run on 8 cores; in_cat maps name -> concatenated [8*shard] array.
    Returns the single output reshaped [8, TQ, D]."""
    try:
        import jax
        from jax.sharding import PartitionSpec, NamedSharding
        if "runner" not in _PROG_CACHE:
            _PROG_CACHE["runner"] = _make_runner(nc)
        fn, in_names, out_names, out_avals, mesh = _PROG_CACHE["runner"]
        sh = NamedSharding(mesh, PartitionSpec("core"))
        args = [jax.device_put(np.ascontiguousarray(in_cat[nm]), sh)
                for nm in in_names]
        if "zeros_out" not in _PROG_CACHE:
            _PROG_CACHE["zeros_out"] = [
                jax.device_put(np.zeros((8 * a.shape[0], *a.shape[1:]), a.dtype), sh)
                for a in out_avals]
        args += _PROG_CACHE["zeros_out"]
        outs = jax.block_until_ready(fn(*args))
        return np.asarray(outs[0]).reshape(8, TQ, D)
    except Exception:
        from concourse.bass_utils import run_bass_kernel_spmd
        global LAST_RESULT
        shard0 = {"xs": D, "wh": 3 * D // 2, "bias": 3, "wt": T}
        in_maps = [{nm: in_cat[nm][c * n0:(c + 1) * n0]
                    for nm, n0 in shard0.items()} for c in range(8)]
        res = run_bass_kernel_spmd(nc, in_maps, core_ids=list(range(8)),
                                   trace=trace)
        LAST_RESULT = res
        return np.stack([res.results[c]["out"] for c in range(8)])
